# revision 32
# baseline (speedup 1.0000x reference)
"""Trainium2 Bass kernel for a dense transformer block (B=8, S=1024, D=768,
H=12 heads, FF=3072), data-parallel over batch across 8 NeuronCores.

Layout: all activations transposed on chip ([D, S], features on partitions)
so weight matrices (stored [in, out]) are directly the matmul stationary
operand and per-feature biases are per-partition on ACT evictions.  Attention
scores are computed transposed ([k, q]); the softmax denominator comes free
from a ones-column appended to V.  The host transposes x on the way in and
the outputs on the way out.

Matmul operands are fp16 (PE full rate; fp32-family matmuls are limited to a
single semaphore wait which Tile cannot guarantee).  PSUM accumulation,
softmax scales, LN statistics and the residual adds stay fp32.
Cross-partition broadcasts run on the otherwise idle GPSIMD engine.

All multi-chunk activations are stored as per-chunk SBUF tiles because the
Tile framework tracks dependencies per tile: chunked tiles let consumers
start as soon as their chunk is ready instead of waiting for the whole
tensor.
"""

import os
import numpy as np

D = 768
H = 12
DH = 64
S = 1024
FF = 3072
EPS = 1e-6
P = 128
KC = D // P      # 6 chunks of the hidden dim
MC_FF = FF // P  # 24 chunks of the mlp dim
KT = S // P      # 8 tiles of the sequence (k) dim
B = 8

PACK_SCORES = os.environ.get("KERNEL_PACK_SCORES", "1") == "1"

_CACHE = {}


def _build():
    import concourse.tile as tile
    from concourse import bacc, mybir

    f32 = mybir.dt.float32
    f16 = mybir.dt.float16
    AF = mybir.ActivationFunctionType

    nc = bacc.Bacc("TRN2")
    mm = nc.tensor.matmul

    # ---------------- DRAM I/O ----------------
    xTf = nc.dram_tensor("xTf", [D, S], f32, kind="ExternalInput")
    xTb = nc.dram_tensor("xTb", [D, S], f16, kind="ExternalInput")
    wq = nc.dram_tensor("wq", [D, D], f16, kind="ExternalInput")
    wk = nc.dram_tensor("wk", [D, D], f16, kind="ExternalInput")
    wv = nc.dram_tensor("wv", [D, D], f16, kind="ExternalInput")
    wo = nc.dram_tensor("wo", [D, D], f16, kind="ExternalInput")
    bq = nc.dram_tensor("bq", [D], f32, kind="ExternalInput")
    bk = nc.dram_tensor("bk", [D], f32, kind="ExternalInput")
    bv = nc.dram_tensor("bv", [D], f32, kind="ExternalInput")
    bo = nc.dram_tensor("bo", [D], f32, kind="ExternalInput")
    w1 = nc.dram_tensor("w1", [D, FF], f16, kind="ExternalInput")
    b1 = nc.dram_tensor("b1", [FF], f32, kind="ExternalInput")
    w2 = nc.dram_tensor("w2", [FF, D], f16, kind="ExternalInput")
    b2 = nc.dram_tensor("b2", [D], f32, kind="ExternalInput")
    ln1_g = nc.dram_tensor("ln1_g", [D], f32, kind="ExternalInput")
    ln1_b = nc.dram_tensor("ln1_b", [D], f32, kind="ExternalInput")
    ln2_g = nc.dram_tensor("ln2_g", [D], f32, kind="ExternalInput")
    ln2_b = nc.dram_tensor("ln2_b", [D], f32, kind="ExternalInput")

    outT = nc.dram_tensor("outT", [D, S], f32, kind="ExternalOutput")
    wT = nc.dram_tensor("wT", [H, S, S], f16, kind="ExternalOutput")

    def qsl(qh, w=512):
        return slice(qh * w, (qh + 1) * w)

    def csl(c, w=128):
        return slice(c * w, (c + 1) * w)

    with tile.TileContext(nc) as tc, \
         tc.tile_pool(name="const", bufs=1) as const_pool, \
         tc.tile_pool(name="rows", bufs=1) as rows_pool, \
         tc.tile_pool(name="recip", bufs=2) as recip_pool, \
         tc.tile_pool(name="sqt", bufs=3) as sqt_pool, \
         tc.tile_pool(name="bcast", bufs=2) as bcast_pool, \
         tc.tile_pool(name="evt", bufs=3) as evt_pool, \
         tc.tile_pool(name="ctxt", bufs=1) as ctxt_pool, \
         tc.tile_pool(name="wop", bufs=1) as wo_pool:

        ones_red = const_pool.tile([P, 1], f16)
        nc.vector.memset(ones_red[:], 1.0)
        eps_col = const_pool.tile([1, 1], f32)
        nc.vector.memset(eps_col[:], float(EPS))

        # ctx^T chunks (written per head pair, read by out-proj)
        ctxT_c = [ctxt_pool.tile([P, S], f16, tag=f"ctx{i}", name=f"ctx{i}")
                  for i in range(KC)]
        wo_sb = wo_pool.tile([P, KC, D], f16)
        for kc in range(KC):
            nc.sync.dma_start(wo_sb[:, kc, :], wo[csl(kc), :])

        # ---- layernorm over the feature (partition x chunk) dim, one
        # q-half at a time so consumers of the first half start early.
        # srcb(kc, qh)/srcf(kc, qh)/dst(kc, qh) -> [P, 512] APs.
        def ln_transposed(srcb, srcf, dst, g_dram, b_dram, tag):
            gb_sb = const_pool.tile([P, KC, 2], f32, tag=f"gb{tag}")
            nc.sync.dma_start(gb_sb[:, :, 0],
                              g_dram.rearrange("(c p) -> p c", p=P))
            nc.sync.dma_start(gb_sb[:, :, 1],
                              b_dram.rearrange("(c p) -> p c", p=P))

            with tc.tile_pool(name="lnstats", bufs=2, space="PSUM") as ps_st:
                for qh in range(2):
                    sx = ps_st.tile([1, 512], f32, tag="st",
                                    name=f"sx{tag}{qh}")
                    sq = ps_st.tile([1, 512], f32, tag="st",
                                    name=f"sq{tag}{qh}")
                    for kc in range(KC):
                        t = sqt_pool.tile([P, 512], f16, tag="sq16")
                        nc.vector.tensor_mul(t[:], srcb(kc, qh), srcb(kc, qh))
                        mm(sx[:], ones_red[:], srcb(kc, qh),
                           start=(kc == 0), stop=(kc == KC - 1))
                        mm(sq[:], ones_red[:], t[:],
                           start=(kc == 0), stop=(kc == KC - 1))
                    mu = rows_pool.tile([1, 512], f32, tag="mu", bufs=2)
                    nc.scalar.activation(mu[:], sx[:], AF.Copy, scale=1.0 / D)
                    ex2 = rows_pool.tile([1, 512], f32, tag="ex2", bufs=2)
                    nc.scalar.activation(ex2[:], sq[:], AF.Copy,
                                         scale=1.0 / D)
                    scr = rows_pool.tile([1, 512], f32, tag="scr", bufs=2)
                    nc.vector.tensor_mul(scr[:], mu[:], mu[:])
                    nc.vector.tensor_sub(ex2[:], ex2[:], scr[:])
                    nc.scalar.activation(ex2[:], ex2[:], AF.Sqrt,
                                         bias=eps_col[:])
                    rstd = rows_pool.tile([1, 512], f32, tag="rstd", bufs=2)
                    nc.vector.reciprocal(rstd[:], ex2[:])

                    mu_b = bcast_pool.tile([P, 512], f32, tag="mu_b")
                    nc.gpsimd.partition_broadcast(mu_b[:], mu[:])
                    rstd_b = bcast_pool.tile([P, 512], f32, tag="rstd_b")
                    nc.gpsimd.partition_broadcast(rstd_b[:], rstd[:])

                    for kc in range(KC):
                        eng = nc.vector if kc < 4 else nc.gpsimd
                        t = sqt_pool.tile([P, 512], f32, tag="nrm")
                        eng.tensor_sub(t[:], srcf(kc, qh), mu_b[:])
                        eng.tensor_mul(t[:], t[:], rstd_b[:])
                        eng.tensor_scalar(
                            out=dst(kc, qh), in0=t[:],
                            scalar1=gb_sb[:, kc, 0:1],
                            scalar2=gb_sb[:, kc, 1:2],
                            op0=mybir.AluOpType.mult,
                            op1=mybir.AluOpType.add)

        # =================== Phase A: LN1 + QKV ===================
        with tc.tile_pool(name="qt", bufs=1) as qt_pool, \
             tc.tile_pool(name="kt", bufs=1) as kt_pool, \
             tc.tile_pool(name="vt", bufs=1) as vt_pool:
            QT_c = [qt_pool.tile([P, S], f16, tag=f"q{i}", name=f"q{i}")
                    for i in range(KC)]
            KT_c = [kt_pool.tile([P, S], f16, tag=f"k{i}", name=f"k{i}")
                    for i in range(KC)]
            V_c = [vt_pool.tile([P, H, DH + 1], f16, tag=f"v{i}",
                                name=f"v{i}")
                   for i in range(KT)]

            with tc.tile_pool(name="xn1", bufs=1) as xn1_pool:
                xn_qc = [[xn1_pool.tile([P, 512], f16, tag=f"xn{i}_{qh}",
                                        name=f"xn{i}_{qh}")
                          for qh in range(2)] for i in range(KC)]
                with tc.tile_pool(name="xt", bufs=1) as xt_pool:
                    xTf_c = []
                    xTb_c = []
                    for kc in range(KC):
                        tf = xt_pool.tile([P, S], f32, tag=f"xf{kc}",
                                          name=f"xf{kc}")
                        nc.sync.dma_start(tf[:], xTf[csl(kc), :])
                        xTf_c.append(tf)
                        tb = xt_pool.tile([P, S], f16, tag=f"xb{kc}",
                                          name=f"xb{kc}")
                        nc.sync.dma_start(tb[:], xTb[csl(kc), :])
                        xTb_c.append(tb)
                    ln_transposed(
                        lambda kc, qh: xTb_c[kc][:, qh * 512:qh * 512 + 512],
                        lambda kc, qh: xTf_c[kc][:, qh * 512:qh * 512 + 512],
                        lambda kc, qh: xn_qc[kc][qh][:],
                        ln1_g, ln1_b, "1")

                # --- V (natural layout + ones column)
                with tc.tile_pool(name="wv", bufs=1) as wv_pool:
                    wv_sb = wv_pool.tile([P, KC, D], f16)
                    for kc in range(KC):
                        nc.sync.dma_start(wv_sb[:, kc, :], wv[csl(kc), :])
                    bv_row = rows_pool.tile([1, D], f32, tag="bvr")
                    nc.sync.dma_start(bv_row[:], bv[None, :])
                    bv_bb = const_pool.tile([P, D], f32, tag="bvb")
                    nc.gpsimd.partition_broadcast(bv_bb[:], bv_row[:])
                    with tc.tile_pool(name="vps", bufs=4,
                                      space="PSUM") as ps_v:
                        for st in range(KT):
                            for off, n in ((0, 512), (512, 256)):
                                ps = ps_v.tile([P, 512], f32, tag="mm")
                                for kc in range(KC):
                                    mm(ps[:, :n],
                                   xn_qc[kc][st // 4][:, csl(st % 4)],
                                       wv_sb[:, kc, off:off + n],
                                       start=(kc == 0), stop=(kc == KC - 1))
                                h0, nh = off // DH, n // DH
                                nc.vector.tensor_add(
                                    V_c[st][:, h0:h0 + nh, 0:DH],
                                    ps[:, :n].rearrange("p (h c) -> p h c",
                                                        c=DH),
                                    bv_bb[:, off:off + n].rearrange(
                                        "p (h c) -> p h c", c=DH))
                            nc.vector.memset(V_c[st][:, :, DH:DH + 1], 1.0)

                # --- Q^T, K^T with streamed weight column blocks
                bq_sb = const_pool.tile([P, KC], f32, tag="bq")
                bk_sb = const_pool.tile([P, KC], f32, tag="bk")
                nc.sync.dma_start(bq_sb[:], bq.rearrange("(c p) -> p c", p=P))
                nc.sync.dma_start(bk_sb[:], bk.rearrange("(c p) -> p c", p=P))
                with tc.tile_pool(name="wstream", bufs=3) as ws_pool, \
                     tc.tile_pool(name="qkps", bufs=4, space="PSUM") as ps_qk:
                    for mc in range(KC):
                        for w_dram, b_sb, dstC in ((wq, bq_sb, QT_c),
                                                   (wk, bk_sb, KT_c)):
                            wcol = ws_pool.tile([P, KC, P], f16, tag="wcol")
                            nc.sync.dma_start(
                                wcol[:],
                                w_dram.rearrange("(c p) m -> p c m",
                                                 p=P)[:, :, csl(mc)])
                            for qh in range(2):
                                ps = ps_qk.tile([P, 512], f32, tag="mm")
                                for kc in range(KC):
                                    mm(ps[:], wcol[:, kc, :],
                                       xn_qc[kc][qh][:],
                                       start=(kc == 0), stop=(kc == KC - 1))
                                nc.scalar.activation(
                                    dstC[mc][:, qsl(qh)], ps[:], AF.Identity,
                                    bias=b_sb[:, mc:mc + 1])

            # =================== Phase B: attention ===================
            with tc.tile_pool(name="wexp", bufs=2) as wexp_pool, \
                 tc.tile_pool(name="wstage", bufs=6) as wstage_pool, \
                 tc.tile_pool(name="scps", bufs=2, space="PSUM") as ps_sc, \
                 tc.tile_pool(name="cxps", bufs=2, space="PSUM") as ps_cx:
                for h in range(H):
                    ch, off = h // 2, (h % 2) * DH
                    we_c = []
                    for kt in range(KT):
                        sc = ps_sc.tile([P, S], f32, tag="sc")
                        for qh in range(2):
                            kw = {}
                            if PACK_SCORES:
                                kw["tile_position"] = (off, 0)
                            mm(sc[:, qsl(qh)],
                               KT_c[ch][off:off + DH, csl(kt)],
                               QT_c[ch][off:off + DH, qsl(qh)],
                               start=True, stop=True, **kw)
                        we = wexp_pool.tile([P, S], f16, tag=f"we{kt}",
                                            name=f"we{h}_{kt}")
                        # exp(scores/8); max-subtraction skipped (scores O(1))
                        nc.scalar.activation(we[:], sc[:], AF.Exp, scale=0.125)
                        we_c.append(we)
                    cx = ps_cx.tile([DH + 1, S], f32, tag="cx")
                    for kt in range(KT):
                        for qh in range(2):
                            mm(cx[:, qsl(qh)], V_c[kt][:, h, :],
                               we_c[kt][:, qsl(qh)],
                               start=(kt == 0), stop=(kt == KT - 1))
                    rr = recip_pool.tile([1, S], f16, tag="rr")
                    with nc.allow_low_precision(reason="fp16 probs output"):
                        nc.vector.reciprocal(rr[:], cx[DH:DH + 1, :])
                    rb = bcast_pool.tile([P, S], f16, tag="rb")
                    nc.gpsimd.partition_broadcast(rb[:], rr[:])
                    nc.vector.tensor_mul(ctxT_c[ch][off:off + DH, :],
                                         cx[0:DH, :], rb[0:DH, :])
                    for kt in range(KT):
                        weng = nc.gpsimd if kt in (2, 5, 7) else nc.vector
                        wn = wstage_pool.tile([P, S], f16, tag="wn")
                        weng.tensor_mul(wn[:], we_c[kt][:], rb[:])
                        nc.sync.dma_start(wT[h, csl(kt), :], wn[:])

        # =============== Phase C: out-proj + residual + LN2 ===============
        with tc.tile_pool(name="x2t", bufs=1) as x2t_pool:
            x2f_qc = [[x2t_pool.tile([P, 512], f32, tag=f"x2f{i}_{qh}",
                                     name=f"x2f{i}_{qh}") for qh in range(2)]
                      for i in range(KC)]
            x2b_qc = [[x2t_pool.tile([P, 512], f16, tag=f"x2b{i}_{qh}",
                                     name=f"x2b{i}_{qh}") for qh in range(2)]
                      for i in range(KC)]
            bo_sb = const_pool.tile([P, KC], f32, tag="bo")
            nc.sync.dma_start(bo_sb[:], bo.rearrange("(c p) -> p c", p=P))
            with tc.tile_pool(name="xt2", bufs=1) as xt2_pool, \
                 tc.tile_pool(name="ops", bufs=2, space="PSUM") as ps_o:
                xT2_c = []
                for kc in range(KC):
                    t2 = xt2_pool.tile([P, S], f32, tag=f"xr{kc}",
                                       name=f"xr{kc}")
                    nc.gpsimd.dma_start(t2[:], xTf[csl(kc), :])
                    xT2_c.append(t2)
                for mc in range(KC):
                    for qh in range(2):
                        ps = ps_o.tile([P, 512], f32, tag="o")
                        for kc in range(KC):
                            mm(ps[:], wo_sb[:, kc, csl(mc)],
                               ctxT_c[kc][:, qsl(qh)],
                               start=(kc == 0), stop=(kc == KC - 1))
                        t = evt_pool.tile([P, 512], f32, tag="ev")
                        nc.scalar.activation(t[:], ps[:], AF.Identity,
                                             bias=bo_sb[:, mc:mc + 1])
                        nc.vector.tensor_add(x2f_qc[mc][qh][:], t[:],
                                             xT2_c[mc][:, qsl(qh)])
                        nc.scalar.activation(x2b_qc[mc][qh][:],
                                             x2f_qc[mc][qh][:], AF.Copy)

            # ====================== Phase D: MLP ======================
            with tc.tile_pool(name="xn2", bufs=1) as xn2_pool:
                xn2_qc = [[xn2_pool.tile([P, 512], f16, tag=f"n2{i}_{qh}",
                                         name=f"n2{i}_{qh}")
                           for qh in range(2)] for i in range(KC)]
                ln_transposed(
                    lambda kc, qh: x2b_qc[kc][qh][:],
                    lambda kc, qh: x2f_qc[kc][qh][:],
                    lambda kc, qh: xn2_qc[kc][qh][:],
                    ln2_g, ln2_b, "2")

                b1_sb = const_pool.tile([P, MC_FF], f32, tag="b1")
                nc.sync.dma_start(b1_sb[:], b1.rearrange("(c p) -> p c", p=P))
                b2_sb = const_pool.tile([P, KC], f32, tag="b2")
                nc.sync.dma_start(b2_sb[:], b2.rearrange("(c p) -> p c", p=P))

                with tc.tile_pool(name="w1s", bufs=3) as w1s_pool, \
                     tc.tile_pool(name="w2s", bufs=3) as w2s_pool, \
                     tc.tile_pool(name="hg", bufs=4) as hg_pool, \
                     tc.tile_pool(name="ostage", bufs=4) as ostage_pool, \
                     tc.tile_pool(name="fc1ps", bufs=2,
                                  space="PSUM") as ps_f1, \
                     tc.tile_pool(name="fc2ps", bufs=6,
                                  space="PSUM") as ps_f2:
                    for qh in range(2):
                        ps2 = [ps_f2.tile([P, 512], f32, tag="fc2",
                                          name=f"fc2_{qh}_{i}")
                               for i in range(KC)]
                        for mc in range(MC_FF):
                            w1col = w1s_pool.tile([P, KC, P], f16, tag="w1c")
                            nc.sync.dma_start(
                                w1col[:],
                                w1.rearrange("(c p) m -> p c m",
                                             p=P)[:, :, csl(mc)])
                            w2t = w2s_pool.tile([P, D], f16, tag="w2t")
                            nc.sync.dma_start(w2t[:], w2[csl(mc), :])
                            ps1 = ps_f1.tile([P, 512], f32, tag="fc1")
                            for kc in range(KC):
                                mm(ps1[:], w1col[:, kc, :],
                                   xn2_qc[kc][qh][:],
                                   start=(kc == 0), stop=(kc == KC - 1))
                            hg = hg_pool.tile([P, 512], f16, tag="hg")
                            nc.scalar.activation(hg[:], ps1[:], AF.Gelu,
                                                 bias=b1_sb[:, mc:mc + 1])
                            for mc2 in range(KC):
                                mm(ps2[mc2][:], w2t[:, csl(mc2)], hg[:],
                                   start=(mc == 0), stop=(mc == MC_FF - 1))
                        for mc2 in range(KC):
                            t = evt_pool.tile([P, 512], f32, tag="ev")
                            nc.scalar.activation(t[:], ps2[mc2][:],
                                                 AF.Identity,
                                                 bias=b2_sb[:, mc2:mc2 + 1])
                            ot = ostage_pool.tile([P, 512], f32, tag="ot")
                            nc.vector.tensor_add(ot[:], t[:],
                                                 x2f_qc[mc2][qh][:])
                            nc.sync.dma_start(outT[csl(mc2), qsl(qh)], ot[:])

    nc.compile()
    return nc


def _get_nc():
    key = PACK_SCORES
    if key not in _CACHE:
        _CACHE[key] = _build()
    return _CACHE[key]


def kernel(**inputs):
    from concourse.bass_utils import run_bass_kernel_spmd

    nc = _get_nc()
    x = np.asarray(inputs["x"], dtype=np.float32)
    shared = {}
    for name in ("bq", "bk", "bv", "bo", "b1", "b2",
                 "ln1_g", "ln1_b", "ln2_g", "ln2_b"):
        shared[name] = np.ascontiguousarray(
            np.asarray(inputs[name], dtype=np.float32))
    for name in ("wq", "wk", "wv", "wo", "w1", "w2"):
        shared[name] = np.ascontiguousarray(
            np.asarray(inputs[name], dtype=np.float32).astype(np.float16))
    in_maps = []
    for b in range(B):
        xt = np.ascontiguousarray(x[b].T)
        m = dict(shared)
        m["xTf"] = xt
        m["xTb"] = xt.astype(np.float16)
        in_maps.append(m)

    res = run_bass_kernel_spmd(nc, in_maps, core_ids=list(range(B)))
    globals()["_LAST_RESULT"] = res  # for test.py profiling
    out = np.stack([r["outT"].T for r in res.results])
    probs = np.stack([r["wT"].transpose(0, 2, 1).astype(np.float32)
                      for r in res.results])
    return np.ascontiguousarray(out), np.ascontiguousarray(probs)


# revision 33
# speedup vs baseline: 1.0030x; 1.0030x over previous
"""Trainium2 Bass kernel for a dense transformer block (B=8, S=1024, D=768,
H=12 heads, FF=3072), data-parallel over batch across 8 NeuronCores.

Layout: all activations transposed on chip ([D, S], features on partitions)
so weight matrices (stored [in, out]) are directly the matmul stationary
operand and per-feature biases are per-partition on ACT evictions.  Attention
scores are computed transposed ([k, q]); the softmax denominator comes free
from a ones-column appended to V.  The host transposes x on the way in and
the outputs on the way out.

Matmul operands are fp16 (PE full rate; fp32-family matmuls are limited to a
single semaphore wait which Tile cannot guarantee).  PSUM accumulation,
softmax scales, LN statistics and the residual adds stay fp32.
Cross-partition broadcasts run on the otherwise idle GPSIMD engine.

All multi-chunk activations are stored as per-chunk SBUF tiles because the
Tile framework tracks dependencies per tile: chunked tiles let consumers
start as soon as their chunk is ready instead of waiting for the whole
tensor.
"""

import os
import numpy as np

D = 768
H = 12
DH = 64
S = 1024
FF = 3072
EPS = 1e-6
P = 128
KC = D // P      # 6 chunks of the hidden dim
MC_FF = FF // P  # 24 chunks of the mlp dim
KT = S // P      # 8 tiles of the sequence (k) dim
B = 8

PACK_SCORES = os.environ.get("KERNEL_PACK_SCORES", "1") == "1"

_CACHE = {}


def _build():
    import concourse.tile as tile
    from concourse import bacc, mybir

    f32 = mybir.dt.float32
    f16 = mybir.dt.float16
    AF = mybir.ActivationFunctionType

    nc = bacc.Bacc("TRN2")
    mm = nc.tensor.matmul

    # ---------------- DRAM I/O ----------------
    xTf = nc.dram_tensor("xTf", [D, S], f32, kind="ExternalInput")
    xTb = nc.dram_tensor("xTb", [D, S], f16, kind="ExternalInput")
    wq = nc.dram_tensor("wq", [D, D], f16, kind="ExternalInput")
    wk = nc.dram_tensor("wk", [D, D], f16, kind="ExternalInput")
    wv = nc.dram_tensor("wv", [D, D], f16, kind="ExternalInput")
    wo = nc.dram_tensor("wo", [D, D], f16, kind="ExternalInput")
    bq = nc.dram_tensor("bq", [D], f32, kind="ExternalInput")
    bk = nc.dram_tensor("bk", [D], f32, kind="ExternalInput")
    bv = nc.dram_tensor("bv", [D], f32, kind="ExternalInput")
    bo = nc.dram_tensor("bo", [D], f32, kind="ExternalInput")
    w1 = nc.dram_tensor("w1", [D, FF], f16, kind="ExternalInput")
    b1 = nc.dram_tensor("b1", [FF], f32, kind="ExternalInput")
    w2 = nc.dram_tensor("w2", [FF, D], f16, kind="ExternalInput")
    b2 = nc.dram_tensor("b2", [D], f32, kind="ExternalInput")
    ln1_g = nc.dram_tensor("ln1_g", [D], f32, kind="ExternalInput")
    ln1_b = nc.dram_tensor("ln1_b", [D], f32, kind="ExternalInput")
    ln2_g = nc.dram_tensor("ln2_g", [D], f32, kind="ExternalInput")
    ln2_b = nc.dram_tensor("ln2_b", [D], f32, kind="ExternalInput")

    outT = nc.dram_tensor("outT", [D, S], f32, kind="ExternalOutput")
    wT = nc.dram_tensor("wT", [H, S, S], f16, kind="ExternalOutput")

    def qsl(qh, w=512):
        return slice(qh * w, (qh + 1) * w)

    def csl(c, w=128):
        return slice(c * w, (c + 1) * w)

    with tile.TileContext(nc) as tc, \
         tc.tile_pool(name="const", bufs=1) as const_pool, \
         tc.tile_pool(name="rows", bufs=1) as rows_pool, \
         tc.tile_pool(name="recip", bufs=2) as recip_pool, \
         tc.tile_pool(name="sqt", bufs=3) as sqt_pool, \
         tc.tile_pool(name="bcast", bufs=2) as bcast_pool, \
         tc.tile_pool(name="evt", bufs=3) as evt_pool, \
         tc.tile_pool(name="ctxt", bufs=1) as ctxt_pool, \
         tc.tile_pool(name="wop", bufs=1) as wo_pool:

        ones_red = const_pool.tile([P, 1], f16)
        nc.vector.memset(ones_red[:], 1.0)
        eps_col = const_pool.tile([1, 1], f32)
        nc.vector.memset(eps_col[:], float(EPS))

        # ctx^T chunks (written per head pair, read by out-proj)
        ctxT_c = [ctxt_pool.tile([P, S], f16, tag=f"ctx{i}", name=f"ctx{i}")
                  for i in range(KC)]
        wo_sb = wo_pool.tile([P, KC, D], f16)
        for kc in range(KC):
            nc.sync.dma_start(wo_sb[:, kc, :], wo[csl(kc), :])

        # ---- layernorm over the feature (partition x chunk) dim, one
        # q-half at a time so consumers of the first half start early.
        # srcb(kc, qh)/srcf(kc, qh)/dst(kc, qh) -> [P, 512] APs.
        def ln_transposed(srcb, srcf, dst, g_dram, b_dram, tag):
            gb_sb = const_pool.tile([P, KC, 2], f32, tag=f"gb{tag}")
            nc.sync.dma_start(gb_sb[:, :, 0],
                              g_dram.rearrange("(c p) -> p c", p=P))
            nc.sync.dma_start(gb_sb[:, :, 1],
                              b_dram.rearrange("(c p) -> p c", p=P))

            with tc.tile_pool(name="lnstats", bufs=2, space="PSUM") as ps_st:
                for qh in range(2):
                    sx = ps_st.tile([1, 512], f32, tag="st",
                                    name=f"sx{tag}{qh}")
                    sq = ps_st.tile([1, 512], f32, tag="st",
                                    name=f"sq{tag}{qh}")
                    for kc in range(KC):
                        t = sqt_pool.tile([P, 512], f16, tag="sq16")
                        nc.vector.tensor_mul(t[:], srcb(kc, qh), srcb(kc, qh))
                        mm(sx[:], ones_red[:], srcb(kc, qh),
                           start=(kc == 0), stop=(kc == KC - 1))
                        mm(sq[:], ones_red[:], t[:],
                           start=(kc == 0), stop=(kc == KC - 1))
                    mu = rows_pool.tile([1, 512], f32, tag="mu", bufs=2)
                    nc.scalar.activation(mu[:], sx[:], AF.Copy, scale=1.0 / D)
                    ex2 = rows_pool.tile([1, 512], f32, tag="ex2", bufs=2)
                    nc.scalar.activation(ex2[:], sq[:], AF.Copy,
                                         scale=1.0 / D)
                    scr = rows_pool.tile([1, 512], f32, tag="scr", bufs=2)
                    nc.vector.tensor_mul(scr[:], mu[:], mu[:])
                    nc.vector.tensor_sub(ex2[:], ex2[:], scr[:])
                    nc.scalar.activation(ex2[:], ex2[:], AF.Sqrt,
                                         bias=eps_col[:])
                    rstd = rows_pool.tile([1, 512], f32, tag="rstd", bufs=2)
                    nc.vector.reciprocal(rstd[:], ex2[:])

                    mu_b = bcast_pool.tile([P, 512], f32, tag="mu_b")
                    nc.gpsimd.partition_broadcast(mu_b[:], mu[:])
                    rstd_b = bcast_pool.tile([P, 512], f32, tag="rstd_b")
                    nc.gpsimd.partition_broadcast(rstd_b[:], rstd[:])

                    for kc in range(KC):
                        eng = nc.vector if kc < 4 else nc.gpsimd
                        t = sqt_pool.tile([P, 512], f32, tag="nrm")
                        eng.tensor_sub(t[:], srcf(kc, qh), mu_b[:])
                        eng.tensor_mul(t[:], t[:], rstd_b[:])
                        eng.tensor_scalar(
                            out=dst(kc, qh), in0=t[:],
                            scalar1=gb_sb[:, kc, 0:1],
                            scalar2=gb_sb[:, kc, 1:2],
                            op0=mybir.AluOpType.mult,
                            op1=mybir.AluOpType.add)

        # =================== Phase A: LN1 + QKV ===================
        with tc.tile_pool(name="qt", bufs=1) as qt_pool, \
             tc.tile_pool(name="kt", bufs=1) as kt_pool, \
             tc.tile_pool(name="vt", bufs=1) as vt_pool:
            QT_c = [qt_pool.tile([P, S], f16, tag=f"q{i}", name=f"q{i}")
                    for i in range(KC)]
            KT_c = [kt_pool.tile([P, S], f16, tag=f"k{i}", name=f"k{i}")
                    for i in range(KC)]
            V_c = [vt_pool.tile([P, H, DH + 1], f16, tag=f"v{i}",
                                name=f"v{i}")
                   for i in range(KT)]

            with tc.tile_pool(name="xn1", bufs=1) as xn1_pool:
                xn_qc = [[xn1_pool.tile([P, 512], f16, tag=f"xn{i}_{qh}",
                                        name=f"xn{i}_{qh}")
                          for qh in range(2)] for i in range(KC)]
                with tc.tile_pool(name="xt", bufs=1) as xt_pool:
                    xTf_c = []
                    xTb_c = []
                    for kc in range(KC):
                        tf = xt_pool.tile([P, S], f32, tag=f"xf{kc}",
                                          name=f"xf{kc}")
                        nc.sync.dma_start(tf[:], xTf[csl(kc), :])
                        xTf_c.append(tf)
                        tb = xt_pool.tile([P, S], f16, tag=f"xb{kc}",
                                          name=f"xb{kc}")
                        nc.sync.dma_start(tb[:], xTb[csl(kc), :])
                        xTb_c.append(tb)
                    ln_transposed(
                        lambda kc, qh: xTb_c[kc][:, qh * 512:qh * 512 + 512],
                        lambda kc, qh: xTf_c[kc][:, qh * 512:qh * 512 + 512],
                        lambda kc, qh: xn_qc[kc][qh][:],
                        ln1_g, ln1_b, "1")

                # --- V (natural layout + ones column)
                with tc.tile_pool(name="wv", bufs=1) as wv_pool:
                    wv_sb = wv_pool.tile([P, KC, D], f16)
                    for kc in range(KC):
                        nc.sync.dma_start(wv_sb[:, kc, :], wv[csl(kc), :])
                    bv_row = rows_pool.tile([1, D], f32, tag="bvr")
                    nc.sync.dma_start(bv_row[:], bv[None, :])
                    bv_bb = const_pool.tile([P, D], f32, tag="bvb")
                    nc.gpsimd.partition_broadcast(bv_bb[:], bv_row[:])
                    with tc.tile_pool(name="vps", bufs=4,
                                      space="PSUM") as ps_v:
                        for st in range(KT):
                            for off, n in ((0, 512), (512, 256)):
                                ps = ps_v.tile([P, 512], f32, tag="mm")
                                for kc in range(KC):
                                    mm(ps[:, :n],
                                   xn_qc[kc][st // 4][:, csl(st % 4)],
                                       wv_sb[:, kc, off:off + n],
                                       start=(kc == 0), stop=(kc == KC - 1))
                                h0, nh = off // DH, n // DH
                                nc.vector.tensor_add(
                                    V_c[st][:, h0:h0 + nh, 0:DH],
                                    ps[:, :n].rearrange("p (h c) -> p h c",
                                                        c=DH),
                                    bv_bb[:, off:off + n].rearrange(
                                        "p (h c) -> p h c", c=DH))
                            nc.vector.memset(V_c[st][:, :, DH:DH + 1], 1.0)

                # --- Q^T, K^T with streamed weight column blocks
                bq_sb = const_pool.tile([P, KC], f32, tag="bq")
                bk_sb = const_pool.tile([P, KC], f32, tag="bk")
                nc.sync.dma_start(bq_sb[:], bq.rearrange("(c p) -> p c", p=P))
                nc.sync.dma_start(bk_sb[:], bk.rearrange("(c p) -> p c", p=P))
                with tc.tile_pool(name="wstream", bufs=3) as ws_pool, \
                     tc.tile_pool(name="qkps", bufs=4, space="PSUM") as ps_qk:
                    for mc in range(KC):
                        for w_dram, b_sb, dstC in ((wq, bq_sb, QT_c),
                                                   (wk, bk_sb, KT_c)):
                            wcol = ws_pool.tile([P, KC, P], f16, tag="wcol")
                            nc.sync.dma_start(
                                wcol[:],
                                w_dram.rearrange("(c p) m -> p c m",
                                                 p=P)[:, :, csl(mc)])
                            for qh in range(2):
                                ps = ps_qk.tile([P, 512], f32, tag="mm")
                                for kc in range(KC):
                                    mm(ps[:], wcol[:, kc, :],
                                       xn_qc[kc][qh][:],
                                       start=(kc == 0), stop=(kc == KC - 1))
                                nc.vector.tensor_scalar(
                                    out=dstC[mc][:, qsl(qh)], in0=ps[:],
                                    scalar1=b_sb[:, mc:mc + 1], scalar2=None,
                                    op0=mybir.AluOpType.add)

            # =================== Phase B: attention ===================
            with tc.tile_pool(name="wexp", bufs=2) as wexp_pool, \
                 tc.tile_pool(name="wstage", bufs=6) as wstage_pool, \
                 tc.tile_pool(name="scps", bufs=2, space="PSUM") as ps_sc, \
                 tc.tile_pool(name="cxps", bufs=2, space="PSUM") as ps_cx:
                for h in range(H):
                    ch, off = h // 2, (h % 2) * DH
                    we_c = []
                    for kt in range(KT):
                        sc = ps_sc.tile([P, S], f32, tag="sc")
                        for qh in range(2):
                            kw = {}
                            if PACK_SCORES:
                                kw["tile_position"] = (off, 0)
                            mm(sc[:, qsl(qh)],
                               KT_c[ch][off:off + DH, csl(kt)],
                               QT_c[ch][off:off + DH, qsl(qh)],
                               start=True, stop=True, **kw)
                        we = wexp_pool.tile([P, S], f16, tag=f"we{kt}",
                                            name=f"we{h}_{kt}")
                        # exp(scores/8); max-subtraction skipped (scores O(1))
                        nc.scalar.activation(we[:], sc[:], AF.Exp, scale=0.125)
                        we_c.append(we)
                    cx = ps_cx.tile([DH + 1, S], f32, tag="cx")
                    for kt in range(KT):
                        for qh in range(2):
                            mm(cx[:, qsl(qh)], V_c[kt][:, h, :],
                               we_c[kt][:, qsl(qh)],
                               start=(kt == 0), stop=(kt == KT - 1))
                    rr = recip_pool.tile([1, S], f16, tag="rr")
                    with nc.allow_low_precision(reason="fp16 probs output"):
                        nc.vector.reciprocal(rr[:], cx[DH:DH + 1, :])
                    rb = bcast_pool.tile([P, S], f16, tag="rb")
                    nc.gpsimd.partition_broadcast(rb[:], rr[:])
                    nc.vector.tensor_mul(ctxT_c[ch][off:off + DH, :],
                                         cx[0:DH, :], rb[0:DH, :])
                    for kt in range(KT):
                        weng = nc.gpsimd if kt in (2, 5, 7) else nc.vector
                        wn = wstage_pool.tile([P, S], f16, tag="wn")
                        weng.tensor_mul(wn[:], we_c[kt][:], rb[:])
                        nc.sync.dma_start(wT[h, csl(kt), :], wn[:])

        # =============== Phase C: out-proj + residual + LN2 ===============
        with tc.tile_pool(name="x2t", bufs=1) as x2t_pool:
            x2f_qc = [[x2t_pool.tile([P, 512], f32, tag=f"x2f{i}_{qh}",
                                     name=f"x2f{i}_{qh}") for qh in range(2)]
                      for i in range(KC)]
            x2b_qc = [[x2t_pool.tile([P, 512], f16, tag=f"x2b{i}_{qh}",
                                     name=f"x2b{i}_{qh}") for qh in range(2)]
                      for i in range(KC)]
            bo_sb = const_pool.tile([P, KC], f32, tag="bo")
            nc.sync.dma_start(bo_sb[:], bo.rearrange("(c p) -> p c", p=P))
            with tc.tile_pool(name="xt2", bufs=1) as xt2_pool, \
                 tc.tile_pool(name="ops", bufs=2, space="PSUM") as ps_o:
                xT2_c = []
                for kc in range(KC):
                    t2 = xt2_pool.tile([P, S], f32, tag=f"xr{kc}",
                                       name=f"xr{kc}")
                    nc.gpsimd.dma_start(t2[:], xTf[csl(kc), :])
                    xT2_c.append(t2)
                for mc in range(KC):
                    for qh in range(2):
                        ps = ps_o.tile([P, 512], f32, tag="o")
                        for kc in range(KC):
                            mm(ps[:], wo_sb[:, kc, csl(mc)],
                               ctxT_c[kc][:, qsl(qh)],
                               start=(kc == 0), stop=(kc == KC - 1))
                        t = evt_pool.tile([P, 512], f32, tag="ev")
                        nc.scalar.activation(t[:], ps[:], AF.Identity,
                                             bias=bo_sb[:, mc:mc + 1])
                        nc.vector.tensor_add(x2f_qc[mc][qh][:], t[:],
                                             xT2_c[mc][:, qsl(qh)])
                        nc.gpsimd.tensor_copy(x2b_qc[mc][qh][:],
                                              x2f_qc[mc][qh][:])

            # ====================== Phase D: MLP ======================
            with tc.tile_pool(name="xn2", bufs=1) as xn2_pool:
                xn2_qc = [[xn2_pool.tile([P, 512], f16, tag=f"n2{i}_{qh}",
                                         name=f"n2{i}_{qh}")
                           for qh in range(2)] for i in range(KC)]
                ln_transposed(
                    lambda kc, qh: x2b_qc[kc][qh][:],
                    lambda kc, qh: x2f_qc[kc][qh][:],
                    lambda kc, qh: xn2_qc[kc][qh][:],
                    ln2_g, ln2_b, "2")

                b1_sb = const_pool.tile([P, MC_FF], f32, tag="b1")
                nc.sync.dma_start(b1_sb[:], b1.rearrange("(c p) -> p c", p=P))
                b2_sb = const_pool.tile([P, KC], f32, tag="b2")
                nc.sync.dma_start(b2_sb[:], b2.rearrange("(c p) -> p c", p=P))

                with tc.tile_pool(name="w1s", bufs=3) as w1s_pool, \
                     tc.tile_pool(name="w2s", bufs=3) as w2s_pool, \
                     tc.tile_pool(name="hg", bufs=4) as hg_pool, \
                     tc.tile_pool(name="ostage", bufs=4) as ostage_pool, \
                     tc.tile_pool(name="fc1ps", bufs=2,
                                  space="PSUM") as ps_f1, \
                     tc.tile_pool(name="fc2ps", bufs=6,
                                  space="PSUM") as ps_f2:
                    for qh in range(2):
                        ps2 = [ps_f2.tile([P, 512], f32, tag="fc2",
                                          name=f"fc2_{qh}_{i}")
                               for i in range(KC)]
                        for mc in range(MC_FF):
                            w1col = w1s_pool.tile([P, KC, P], f16, tag="w1c")
                            nc.sync.dma_start(
                                w1col[:],
                                w1.rearrange("(c p) m -> p c m",
                                             p=P)[:, :, csl(mc)])
                            w2t = w2s_pool.tile([P, D], f16, tag="w2t")
                            nc.sync.dma_start(w2t[:], w2[csl(mc), :])
                            ps1 = ps_f1.tile([P, 512], f32, tag="fc1")
                            for kc in range(KC):
                                mm(ps1[:], w1col[:, kc, :],
                                   xn2_qc[kc][qh][:],
                                   start=(kc == 0), stop=(kc == KC - 1))
                            hg = hg_pool.tile([P, 512], f16, tag="hg")
                            nc.scalar.activation(hg[:], ps1[:], AF.Gelu,
                                                 bias=b1_sb[:, mc:mc + 1])
                            for mc2 in range(KC):
                                mm(ps2[mc2][:], w2t[:, csl(mc2)], hg[:],
                                   start=(mc == 0), stop=(mc == MC_FF - 1))
                        for mc2 in range(KC):
                            t = evt_pool.tile([P, 512], f32, tag="ev")
                            nc.scalar.activation(t[:], ps2[mc2][:],
                                                 AF.Identity,
                                                 bias=b2_sb[:, mc2:mc2 + 1])
                            ot = ostage_pool.tile([P, 512], f32, tag="ot")
                            nc.vector.tensor_add(ot[:], t[:],
                                                 x2f_qc[mc2][qh][:])
                            nc.sync.dma_start(outT[csl(mc2), qsl(qh)], ot[:])

    nc.compile()
    return nc


def _get_nc():
    key = PACK_SCORES
    if key not in _CACHE:
        _CACHE[key] = _build()
    return _CACHE[key]


def kernel(**inputs):
    from concourse.bass_utils import run_bass_kernel_spmd

    nc = _get_nc()
    x = np.asarray(inputs["x"], dtype=np.float32)
    shared = {}
    for name in ("bq", "bk", "bv", "bo", "b1", "b2",
                 "ln1_g", "ln1_b", "ln2_g", "ln2_b"):
        shared[name] = np.ascontiguousarray(
            np.asarray(inputs[name], dtype=np.float32))
    for name in ("wq", "wk", "wv", "wo", "w1", "w2"):
        shared[name] = np.ascontiguousarray(
            np.asarray(inputs[name], dtype=np.float32).astype(np.float16))
    in_maps = []
    for b in range(B):
        xt = np.ascontiguousarray(x[b].T)
        m = dict(shared)
        m["xTf"] = xt
        m["xTb"] = xt.astype(np.float16)
        in_maps.append(m)

    res = run_bass_kernel_spmd(nc, in_maps, core_ids=list(range(B)))
    globals()["_LAST_RESULT"] = res  # for test.py profiling
    out = np.stack([r["outT"].T for r in res.results])
    probs = np.stack([r["wT"].transpose(0, 2, 1).astype(np.float32)
                      for r in res.results])
    return np.ascontiguousarray(out), np.ascontiguousarray(probs)


# revision 35
# speedup vs baseline: 1.0348x; 1.0317x over previous
"""Trainium2 Bass kernel for a dense transformer block (B=8, S=1024, D=768,
H=12 heads, FF=3072), data-parallel over batch across 8 NeuronCores.

Layout: all activations transposed on chip ([D, S], features on partitions)
so weight matrices (stored [in, out]) are directly the matmul stationary
operand and per-feature biases are per-partition on ACT evictions.  Attention
scores are computed transposed ([k, q]); the softmax denominator comes free
from a ones-column appended to V.  The host transposes x on the way in and
the outputs on the way out.

Matmul operands are fp16 (PE full rate; fp32-family matmuls are limited to a
single semaphore wait which Tile cannot guarantee).  PSUM accumulation,
softmax scales, LN statistics and the residual adds stay fp32.
Cross-partition broadcasts run on the otherwise idle GPSIMD engine.

All multi-chunk activations are stored as per-chunk SBUF tiles because the
Tile framework tracks dependencies per tile: chunked tiles let consumers
start as soon as their chunk is ready instead of waiting for the whole
tensor.
"""

import os
import numpy as np

D = 768
H = 12
DH = 64
S = 1024
FF = 3072
EPS = 1e-6
P = 128
KC = D // P      # 6 chunks of the hidden dim
MC_FF = FF // P  # 24 chunks of the mlp dim
KT = S // P      # 8 tiles of the sequence (k) dim
B = 8

PACK_SCORES = os.environ.get("KERNEL_PACK_SCORES", "1") == "1"

_CACHE = {}


def _build():
    import concourse.tile as tile
    from concourse import bacc, mybir

    f32 = mybir.dt.float32
    f16 = mybir.dt.float16
    AF = mybir.ActivationFunctionType

    nc = bacc.Bacc("TRN2")
    mm = nc.tensor.matmul

    # ---------------- DRAM I/O ----------------
    xTf = nc.dram_tensor("xTf", [D, S], f32, kind="ExternalInput")
    xTb = nc.dram_tensor("xTb", [D, S], f16, kind="ExternalInput")
    wq = nc.dram_tensor("wq", [D, D], f16, kind="ExternalInput")
    wk = nc.dram_tensor("wk", [D, D], f16, kind="ExternalInput")
    wv = nc.dram_tensor("wv", [D, D], f16, kind="ExternalInput")
    wo = nc.dram_tensor("wo", [D, D], f16, kind="ExternalInput")
    bq = nc.dram_tensor("bq", [D], f32, kind="ExternalInput")
    bk = nc.dram_tensor("bk", [D], f32, kind="ExternalInput")
    bv = nc.dram_tensor("bv", [D], f32, kind="ExternalInput")
    bo = nc.dram_tensor("bo", [D], f32, kind="ExternalInput")
    w1 = nc.dram_tensor("w1", [D, FF], f16, kind="ExternalInput")
    b1 = nc.dram_tensor("b1", [FF], f32, kind="ExternalInput")
    w2 = nc.dram_tensor("w2", [FF, D], f16, kind="ExternalInput")
    b2 = nc.dram_tensor("b2", [D], f32, kind="ExternalInput")
    ln1_g = nc.dram_tensor("ln1_g", [D], f32, kind="ExternalInput")
    ln1_b = nc.dram_tensor("ln1_b", [D], f32, kind="ExternalInput")
    ln2_g = nc.dram_tensor("ln2_g", [D], f32, kind="ExternalInput")
    ln2_b = nc.dram_tensor("ln2_b", [D], f32, kind="ExternalInput")

    outT = nc.dram_tensor("outT", [D, S], f32, kind="ExternalOutput")
    wT = nc.dram_tensor("wT", [H, S, S], f16, kind="ExternalOutput")

    def qsl(qh, w=512):
        return slice(qh * w, (qh + 1) * w)

    def csl(c, w=128):
        return slice(c * w, (c + 1) * w)

    with tile.TileContext(nc) as tc, \
         tc.tile_pool(name="const", bufs=1) as const_pool, \
         tc.tile_pool(name="rows", bufs=1) as rows_pool, \
         tc.tile_pool(name="recip", bufs=2) as recip_pool, \
         tc.tile_pool(name="sqt", bufs=3) as sqt_pool, \
         tc.tile_pool(name="bcast", bufs=2) as bcast_pool, \
         tc.tile_pool(name="evt", bufs=4) as evt_pool, \
         tc.tile_pool(name="ctxt", bufs=1) as ctxt_pool, \
         tc.tile_pool(name="wop", bufs=1) as wo_pool:

        ones_red = const_pool.tile([P, 1], f16)
        nc.vector.memset(ones_red[:], 1.0)
        eps_col = const_pool.tile([1, 1], f32)
        nc.vector.memset(eps_col[:], float(EPS))

        # ctx^T chunks (written per head pair, read by out-proj)
        ctxT_c = [ctxt_pool.tile([P, S], f16, tag=f"ctx{i}", name=f"ctx{i}")
                  for i in range(KC)]
        wo_sb = wo_pool.tile([P, KC, D], f16)
        for kc in range(KC):
            nc.sync.dma_start(wo_sb[:, kc, :], wo[csl(kc), :])

        # ---- layernorm over the feature (partition x chunk) dim, one
        # q-half at a time so consumers of the first half start early.
        # srcb(kc, qh)/srcf(kc, qh)/dst(kc, qh) -> [P, 512] APs.
        def ln_transposed(srcb, srcf, dst, g_dram, b_dram, tag):
            gb_sb = const_pool.tile([P, KC, 2], f32, tag=f"gb{tag}")
            nc.sync.dma_start(gb_sb[:, :, 0],
                              g_dram.rearrange("(c p) -> p c", p=P))
            nc.sync.dma_start(gb_sb[:, :, 1],
                              b_dram.rearrange("(c p) -> p c", p=P))

            with tc.tile_pool(name="lnstats", bufs=2, space="PSUM") as ps_st:
                for qh in range(2):
                    sx = ps_st.tile([1, 512], f32, tag="st",
                                    name=f"sx{tag}{qh}")
                    sq = ps_st.tile([1, 512], f32, tag="st",
                                    name=f"sq{tag}{qh}")
                    for kc in range(KC):
                        t = sqt_pool.tile([P, 512], f16, tag="sq16")
                        nc.vector.tensor_mul(t[:], srcb(kc, qh), srcb(kc, qh))
                        mm(sx[:], ones_red[:], srcb(kc, qh),
                           start=(kc == 0), stop=(kc == KC - 1))
                        mm(sq[:], ones_red[:], t[:],
                           start=(kc == 0), stop=(kc == KC - 1))
                    mu = rows_pool.tile([1, 512], f32, tag="mu", bufs=2)
                    nc.scalar.activation(mu[:], sx[:], AF.Copy, scale=1.0 / D)
                    ex2 = rows_pool.tile([1, 512], f32, tag="ex2", bufs=2)
                    nc.scalar.activation(ex2[:], sq[:], AF.Copy,
                                         scale=1.0 / D)
                    scr = rows_pool.tile([1, 512], f32, tag="scr", bufs=2)
                    nc.vector.tensor_mul(scr[:], mu[:], mu[:])
                    nc.vector.tensor_sub(ex2[:], ex2[:], scr[:])
                    nc.scalar.activation(ex2[:], ex2[:], AF.Sqrt,
                                         bias=eps_col[:])
                    rstd = rows_pool.tile([1, 512], f32, tag="rstd", bufs=2)
                    nc.vector.reciprocal(rstd[:], ex2[:])

                    mu_b = bcast_pool.tile([P, 512], f32, tag="mu_b")
                    nc.gpsimd.partition_broadcast(mu_b[:], mu[:])
                    rstd_b = bcast_pool.tile([P, 512], f32, tag="rstd_b")
                    nc.gpsimd.partition_broadcast(rstd_b[:], rstd[:])

                    for kc in range(KC):
                        eng = nc.vector if kc < 4 else nc.gpsimd
                        t = sqt_pool.tile([P, 512], f32, tag="nrm")
                        eng.tensor_sub(t[:], srcf(kc, qh), mu_b[:])
                        eng.tensor_mul(t[:], t[:], rstd_b[:])
                        eng.tensor_scalar(
                            out=dst(kc, qh), in0=t[:],
                            scalar1=gb_sb[:, kc, 0:1],
                            scalar2=gb_sb[:, kc, 1:2],
                            op0=mybir.AluOpType.mult,
                            op1=mybir.AluOpType.add)

        # =================== Phase A: LN1 + QKV ===================
        with tc.tile_pool(name="qt", bufs=1) as qt_pool, \
             tc.tile_pool(name="kt", bufs=1) as kt_pool, \
             tc.tile_pool(name="vt", bufs=1) as vt_pool:
            QT_c = [qt_pool.tile([P, S], f16, tag=f"q{i}", name=f"q{i}")
                    for i in range(KC)]
            KT_c = [kt_pool.tile([P, S], f16, tag=f"k{i}", name=f"k{i}")
                    for i in range(KC)]
            V_c = [vt_pool.tile([P, H, DH + 1], f16, tag=f"v{i}",
                                name=f"v{i}")
                   for i in range(KT)]

            with tc.tile_pool(name="xn1", bufs=1) as xn1_pool:
                xn_qc = [[xn1_pool.tile([P, 512], f16, tag=f"xn{i}_{qh}",
                                        name=f"xn{i}_{qh}")
                          for qh in range(2)] for i in range(KC)]
                with tc.tile_pool(name="xt", bufs=1) as xt_pool:
                    xTf_c = []
                    xTb_c = []
                    for kc in range(KC):
                        tf = xt_pool.tile([P, S], f32, tag=f"xf{kc}",
                                          name=f"xf{kc}")
                        nc.sync.dma_start(tf[:], xTf[csl(kc), :])
                        xTf_c.append(tf)
                        tb = xt_pool.tile([P, S], f16, tag=f"xb{kc}",
                                          name=f"xb{kc}")
                        nc.sync.dma_start(tb[:], xTb[csl(kc), :])
                        xTb_c.append(tb)
                    ln_transposed(
                        lambda kc, qh: xTb_c[kc][:, qh * 512:qh * 512 + 512],
                        lambda kc, qh: xTf_c[kc][:, qh * 512:qh * 512 + 512],
                        lambda kc, qh: xn_qc[kc][qh][:],
                        ln1_g, ln1_b, "1")

                # --- V (natural layout + ones column)
                with tc.tile_pool(name="wv", bufs=1) as wv_pool:
                    wv_sb = wv_pool.tile([P, KC, D], f16)
                    for kc in range(KC):
                        nc.sync.dma_start(wv_sb[:, kc, :], wv[csl(kc), :])
                    bv_row = rows_pool.tile([1, D], f32, tag="bvr")
                    nc.sync.dma_start(bv_row[:], bv[None, :])
                    bv_bb = const_pool.tile([P, D], f32, tag="bvb")
                    nc.gpsimd.partition_broadcast(bv_bb[:], bv_row[:])
                    with tc.tile_pool(name="vps", bufs=4,
                                      space="PSUM") as ps_v:
                        for st in range(KT):
                            for off, n in ((0, 512), (512, 256)):
                                ps = ps_v.tile([P, 512], f32, tag="mm")
                                for kc in range(KC):
                                    mm(ps[:, :n],
                                   xn_qc[kc][st // 4][:, csl(st % 4)],
                                       wv_sb[:, kc, off:off + n],
                                       start=(kc == 0), stop=(kc == KC - 1))
                                h0, nh = off // DH, n // DH
                                nc.vector.tensor_add(
                                    V_c[st][:, h0:h0 + nh, 0:DH],
                                    ps[:, :n].rearrange("p (h c) -> p h c",
                                                        c=DH),
                                    bv_bb[:, off:off + n].rearrange(
                                        "p (h c) -> p h c", c=DH))
                            nc.vector.memset(V_c[st][:, :, DH:DH + 1], 1.0)

                # --- Q^T, K^T with streamed weight column blocks
                bq_sb = const_pool.tile([P, KC], f32, tag="bq")
                bk_sb = const_pool.tile([P, KC], f32, tag="bk")
                nc.sync.dma_start(bq_sb[:], bq.rearrange("(c p) -> p c", p=P))
                nc.sync.dma_start(bk_sb[:], bk.rearrange("(c p) -> p c", p=P))
                with tc.tile_pool(name="wstream", bufs=3) as ws_pool, \
                     tc.tile_pool(name="qkps", bufs=4, space="PSUM") as ps_qk:
                    for mc in range(KC):
                        for w_dram, b_sb, dstC in ((wq, bq_sb, QT_c),
                                                   (wk, bk_sb, KT_c)):
                            wcol = ws_pool.tile([P, KC, P], f16, tag="wcol")
                            nc.sync.dma_start(
                                wcol[:],
                                w_dram.rearrange("(c p) m -> p c m",
                                                 p=P)[:, :, csl(mc)])
                            for qh in range(2):
                                ps = ps_qk.tile([P, 512], f32, tag="mm")
                                for kc in range(KC):
                                    mm(ps[:], wcol[:, kc, :],
                                       xn_qc[kc][qh][:],
                                       start=(kc == 0), stop=(kc == KC - 1))
                                nc.vector.tensor_scalar(
                                    out=dstC[mc][:, qsl(qh)], in0=ps[:],
                                    scalar1=b_sb[:, mc:mc + 1], scalar2=None,
                                    op0=mybir.AluOpType.add)

            # =================== Phase B: attention ===================
            with tc.tile_pool(name="wexp", bufs=4) as wexp_pool, \
                 tc.tile_pool(name="wstage", bufs=8) as wstage_pool, \
                 tc.tile_pool(name="scps", bufs=2, space="PSUM") as ps_sc, \
                 tc.tile_pool(name="cxps", bufs=2, space="PSUM") as ps_cx:
                for h in range(H):
                    ch, off = h // 2, (h % 2) * DH
                    we_c = []
                    for kt in range(KT):
                        sc = ps_sc.tile([P, S], f32, tag="sc")
                        for qh in range(2):
                            kw = {}
                            if PACK_SCORES:
                                kw["tile_position"] = (off, 0)
                            mm(sc[:, qsl(qh)],
                               KT_c[ch][off:off + DH, csl(kt)],
                               QT_c[ch][off:off + DH, qsl(qh)],
                               start=True, stop=True, **kw)
                        we = wexp_pool.tile([P, S], f16, tag=f"we{kt}",
                                            name=f"we{h}_{kt}")
                        # exp(scores/8); max-subtraction skipped (scores O(1))
                        nc.scalar.activation(we[:], sc[:], AF.Exp, scale=0.125)
                        we_c.append(we)
                    cx = ps_cx.tile([DH + 1, S], f32, tag="cx")
                    for kt in range(KT):
                        for qh in range(2):
                            mm(cx[:, qsl(qh)], V_c[kt][:, h, :],
                               we_c[kt][:, qsl(qh)],
                               start=(kt == 0), stop=(kt == KT - 1))
                    rr = recip_pool.tile([1, S], f16, tag="rr")
                    with nc.allow_low_precision(reason="fp16 probs output"):
                        nc.vector.reciprocal(rr[:], cx[DH:DH + 1, :])
                    rb = bcast_pool.tile([P, S], f16, tag="rb", bufs=4)
                    nc.gpsimd.partition_broadcast(rb[:], rr[:])
                    nc.vector.tensor_mul(ctxT_c[ch][off:off + DH, :],
                                         cx[0:DH, :], rb[0:DH, :])
                    for kt in range(KT):
                        weng = nc.gpsimd if kt in (2, 5, 7) else nc.vector
                        wn = wstage_pool.tile([P, S], f16, tag="wn")
                        weng.tensor_mul(wn[:], we_c[kt][:], rb[:])
                        nc.sync.dma_start(wT[h, csl(kt), :], wn[:])

        # =============== Phase C: out-proj + residual + LN2 ===============
        with tc.tile_pool(name="x2t", bufs=1) as x2t_pool:
            x2f_qc = [[x2t_pool.tile([P, 512], f32, tag=f"x2f{i}_{qh}",
                                     name=f"x2f{i}_{qh}") for qh in range(2)]
                      for i in range(KC)]
            x2b_qc = [[x2t_pool.tile([P, 512], f16, tag=f"x2b{i}_{qh}",
                                     name=f"x2b{i}_{qh}") for qh in range(2)]
                      for i in range(KC)]
            bo_sb = const_pool.tile([P, KC], f32, tag="bo")
            nc.sync.dma_start(bo_sb[:], bo.rearrange("(c p) -> p c", p=P))
            with tc.tile_pool(name="xt2", bufs=1) as xt2_pool, \
                 tc.tile_pool(name="ops", bufs=2, space="PSUM") as ps_o:
                xT2_c = []
                for kc in range(KC):
                    t2 = xt2_pool.tile([P, S], f32, tag=f"xr{kc}",
                                       name=f"xr{kc}")
                    nc.gpsimd.dma_start(t2[:], xTf[csl(kc), :])
                    xT2_c.append(t2)
                for mc in range(KC):
                    for qh in range(2):
                        ps = ps_o.tile([P, 512], f32, tag="o")
                        for kc in range(KC):
                            mm(ps[:], wo_sb[:, kc, csl(mc)],
                               ctxT_c[kc][:, qsl(qh)],
                               start=(kc == 0), stop=(kc == KC - 1))
                        t = evt_pool.tile([P, 512], f32, tag="ev")
                        nc.scalar.activation(t[:], ps[:], AF.Identity,
                                             bias=bo_sb[:, mc:mc + 1])
                        nc.vector.tensor_add(x2f_qc[mc][qh][:], t[:],
                                             xT2_c[mc][:, qsl(qh)])
                        nc.gpsimd.tensor_copy(x2b_qc[mc][qh][:],
                                              x2f_qc[mc][qh][:])

            # ====================== Phase D: MLP ======================
            with tc.tile_pool(name="xn2", bufs=1) as xn2_pool:
                xn2_qc = [[xn2_pool.tile([P, 512], f16, tag=f"n2{i}_{qh}",
                                         name=f"n2{i}_{qh}")
                           for qh in range(2)] for i in range(KC)]
                ln_transposed(
                    lambda kc, qh: x2b_qc[kc][qh][:],
                    lambda kc, qh: x2f_qc[kc][qh][:],
                    lambda kc, qh: xn2_qc[kc][qh][:],
                    ln2_g, ln2_b, "2")

                b1_sb = const_pool.tile([P, MC_FF], f32, tag="b1")
                nc.sync.dma_start(b1_sb[:], b1.rearrange("(c p) -> p c", p=P))
                b2_sb = const_pool.tile([P, KC], f32, tag="b2")
                nc.sync.dma_start(b2_sb[:], b2.rearrange("(c p) -> p c", p=P))

                with tc.tile_pool(name="w1s", bufs=3) as w1s_pool, \
                     tc.tile_pool(name="w2s", bufs=3) as w2s_pool, \
                     tc.tile_pool(name="hg", bufs=6) as hg_pool, \
                     tc.tile_pool(name="ostage", bufs=6) as ostage_pool, \
                     tc.tile_pool(name="fc1ps", bufs=2,
                                  space="PSUM") as ps_f1, \
                     tc.tile_pool(name="fc2ps", bufs=6,
                                  space="PSUM") as ps_f2:
                    for qh in range(2):
                        ps2 = [ps_f2.tile([P, 512], f32, tag="fc2",
                                          name=f"fc2_{qh}_{i}")
                               for i in range(KC)]
                        for mc in range(MC_FF):
                            w1col = w1s_pool.tile([P, KC, P], f16, tag="w1c")
                            nc.sync.dma_start(
                                w1col[:],
                                w1.rearrange("(c p) m -> p c m",
                                             p=P)[:, :, csl(mc)])
                            w2t = w2s_pool.tile([P, D], f16, tag="w2t")
                            nc.sync.dma_start(w2t[:], w2[csl(mc), :])
                            ps1 = ps_f1.tile([P, 512], f32, tag="fc1")
                            for kc in range(KC):
                                mm(ps1[:], w1col[:, kc, :],
                                   xn2_qc[kc][qh][:],
                                   start=(kc == 0), stop=(kc == KC - 1))
                            hg = hg_pool.tile([P, 512], f16, tag="hg")
                            nc.scalar.activation(hg[:], ps1[:], AF.Gelu,
                                                 bias=b1_sb[:, mc:mc + 1])
                            for mc2 in range(KC):
                                mm(ps2[mc2][:], w2t[:, csl(mc2)], hg[:],
                                   start=(mc == 0), stop=(mc == MC_FF - 1))
                        for mc2 in range(KC):
                            t = evt_pool.tile([P, 512], f32, tag="ev")
                            nc.scalar.activation(t[:], ps2[mc2][:],
                                                 AF.Identity,
                                                 bias=b2_sb[:, mc2:mc2 + 1])
                            ot = ostage_pool.tile([P, 512], f32, tag="ot")
                            nc.vector.tensor_add(ot[:], t[:],
                                                 x2f_qc[mc2][qh][:])
                            nc.sync.dma_start(outT[csl(mc2), qsl(qh)], ot[:])

    nc.compile()
    return nc


def _get_nc():
    key = PACK_SCORES
    if key not in _CACHE:
        _CACHE[key] = _build()
    return _CACHE[key]


def kernel(**inputs):
    from concourse.bass_utils import run_bass_kernel_spmd

    nc = _get_nc()
    x = np.asarray(inputs["x"], dtype=np.float32)
    shared = {}
    for name in ("bq", "bk", "bv", "bo", "b1", "b2",
                 "ln1_g", "ln1_b", "ln2_g", "ln2_b"):
        shared[name] = np.ascontiguousarray(
            np.asarray(inputs[name], dtype=np.float32))
    for name in ("wq", "wk", "wv", "wo", "w1", "w2"):
        shared[name] = np.ascontiguousarray(
            np.asarray(inputs[name], dtype=np.float32).astype(np.float16))
    in_maps = []
    for b in range(B):
        xt = np.ascontiguousarray(x[b].T)
        m = dict(shared)
        m["xTf"] = xt
        m["xTb"] = xt.astype(np.float16)
        in_maps.append(m)

    res = run_bass_kernel_spmd(nc, in_maps, core_ids=list(range(B)))
    globals()["_LAST_RESULT"] = res  # for test.py profiling
    out = np.stack([r["outT"].T for r in res.results])
    probs = np.stack([r["wT"].transpose(0, 2, 1).astype(np.float32)
                      for r in res.results])
    return np.ascontiguousarray(out), np.ascontiguousarray(probs)
